# revision 12
# baseline (speedup 1.0000x reference)
"""Cox proportional-hazards loss (CoxNNet) on 8 Trainium2 NeuronCores.

loss = -mean((theta - log(risk_sum)) * events)
risk_sum[i] = sum_j [d_j >= d_i] * exp(theta_j)        (N = 16384)

Sharding: rows i of the [N, N] risk-set reduction are split across 8 cores
(2048 rows each). Each core receives one flat fp16 "blob" input holding the
replicated full d/theta plus its own row shard:
  [0:16384]      durations (all)
  [16384:32768]  theta (all)
  [32768:34816]  d_i shard
  [34816:36864]  theta_i shard
  [36864:38912]  events_i shard
fp16 wire format: inputs are rounded to fp16 on the host (loss rel-err
~2e-4, vs the 2e-2 gate) and upconverted to fp32 on-device, halving the
bytes over the ~0.1 GB/s axon host link. All comparisons/compute run in
fp32 on exact fp16-valued operands, so the mask is exact for the rounded
inputs.

Per core:
  - loads d, theta in a [128, 128] chunk layout (partition = fast index),
  - w = exp(theta) on the scalar engine,
  - broadcasts its 2048 d_i values across 128 partitions (K=1 matmul),
  - for each of 128 j-chunks: DVE tensor_scalar(is_le) builds the
    [128-j x 2048-i] 0/1 mask, and 4 fp32 matmuls (lhsT = w-chunk [128,1])
    accumulate risk_sum into 4 [1, 512] PSUM banks,
  - epilogue: risk -> ln -> (theta_i - ln) * e_i -> free-dim reduce
    -> one f32 partial per core.
Host combines: loss = -(sum of partials) / N.

Dispatch: the jit(shard_map(bass_exec)) executable is built ONCE and cached;
each call queues the input transfer + launch asynchronously and blocks only
on the tiny [8,1,1] partial fetch (the axon transport has a ~50 ms round-trip
latency, so one sync per call is the whole game).
"""

import numpy as np

import concourse.bass as bass
import concourse.bacc as bacc
import concourse.mybir as mybir
from concourse.tile import TileContext

N = 16384
P = 128
NCH = N // P            # 128 j-chunks per core (all j)
NCORES = 8
NI = N // NCORES        # 2048 i-rows per core
FT = 512                # fp32 moving-operand max / one PSUM bank
NF = NI // FT           # 4 PSUM accumulators

F32 = mybir.dt.float32
F16 = mybir.dt.float16

# gather=True ships only this core's 12 KB shard and AllGathers d/theta
# on-device over NeuronLink; gather=False ships replicated d/theta (76 KB
# per core) and needs no collective.
GATHER = True
BLOB = 3 * NI if GATHER else 2 * N + 3 * NI


def _build(gather: bool = GATHER):
    nc = bacc.Bacc()
    blob_len = 3 * NI if gather else 2 * N + 3 * NI
    blob = nc.declare_dram_parameter("blob", [blob_len], F16, isOutput=False)
    out = nc.declare_dram_parameter("partial", [1, 1], F32, isOutput=True)

    if gather:
        d_sh = blob[0:NI]
        th_sh = blob[NI:2 * NI]
        ev_sh = blob[2 * NI:3 * NI]
    else:
        d_all = blob[0:N]
        th_all = blob[N:2 * N]
        d_sh = blob[2 * N:2 * N + NI]
        th_sh = blob[2 * N + NI:2 * N + 2 * NI]
        ev_sh = blob[2 * N + 2 * NI:2 * N + 3 * NI]

    with TileContext(nc) as tc:
        with (
            tc.tile_pool(name="const", bufs=1) as cpool,
            tc.tile_pool(name="mask", bufs=4) as mpool,
            tc.tile_pool(name="acc", bufs=1, space="PSUM") as ppool,
            tc.tile_pool(name="bc", bufs=2, space="PSUM") as bcpool,
            tc.tile_pool(name="dram", bufs=2, space="DRAM") as dpool,
        ):
            sb_d16 = cpool.tile([P, NCH], F16)   # d[p*128 + c] at [p, c]
            sb_th16 = cpool.tile([P, NCH], F16)
            sb_d = cpool.tile([P, NCH], F32)     # fp32 upconvert (DVE)
            w_act = cpool.tile([P, NCH], F32)    # exp(theta), ACT-written
            w_sb = cpool.tile([P, NCH], F32)     # DVE copy (single-engine deps for PE)
            ones_row = cpool.tile([1, P], F32)   # bcast lhsT [K=1, M=128]
            row_di16 = cpool.tile([1, NI], F16)
            row_thi16 = cpool.tile([1, NI], F16)
            row_ei16 = cpool.tile([1, NI], F16)
            row_di = cpool.tile([1, NI], F32)    # DVE upconverts
            row_thi = cpool.tile([1, NI], F32)
            row_ei = cpool.tile([1, NI], F32)
            bc_di = cpool.tile([P, NI], F32)
            risk_row = cpool.tile([1, NI], F32)
            ln_row = cpool.tile([1, NI], F32)
            diff_row = cpool.tile([1, NI], F32)
            prod_row = cpool.tile([1, NI], F32)
            part_sb = cpool.tile([1, 1], F32)

            # ---- loads (contiguous layouts) ----
            if gather:
                # AllGather [d_sh|th_sh] (8 KB fp16) -> per-core full d/theta
                # over NeuronLink; collectives need DRAM bounce buffers.
                gat_in = dpool.tile([2 * NI], F16)
                gat_out = dpool.tile([NCORES * 2 * NI], F16)
                nc.gpsimd.dma_start(gat_in[:], blob[0:2 * NI])
                nc.gpsimd.collective_compute(
                    "AllGather",
                    mybir.AluOpType.bypass,
                    replica_groups=[list(range(NCORES))],
                    ins=[gat_in.opt()],
                    outs=[gat_out.opt()],
                )
                # gat_out = [d_0|th_0|d_1|th_1|...]; core c's d segment fills
                # partitions [16c, 16c+16) of the chunk layout (idx p*128+ch)
                for c8 in range(NCORES):
                    dseg = gat_out[c8 * 2 * NI: c8 * 2 * NI + NI]
                    tseg = gat_out[c8 * 2 * NI + NI: (c8 + 1) * 2 * NI]
                    nc.sync.dma_start(out=sb_d16[16 * c8:16 * (c8 + 1), :],
                                      in_=dseg.rearrange("(p c) -> p c", p=16))
                    nc.sync.dma_start(out=sb_th16[16 * c8:16 * (c8 + 1), :],
                                      in_=tseg.rearrange("(p c) -> p c", p=16))
            else:
                nc.sync.dma_start(out=sb_d16[:, :], in_=d_all.rearrange("(p c) -> p c", p=P))
                nc.sync.dma_start(out=sb_th16[:, :], in_=th_all.rearrange("(p c) -> p c", p=P))
            nc.sync.dma_start(out=row_di16[:, :], in_=d_sh.rearrange("(o n) -> o n", o=1))
            nc.sync.dma_start(out=row_thi16[:, :], in_=th_sh.rearrange("(o n) -> o n", o=1))
            nc.sync.dma_start(out=row_ei16[:, :], in_=ev_sh.rearrange("(o n) -> o n", o=1))

            # ---- prep: upconvert to fp32, w = exp(theta), broadcast d_i ----
            # PE allows only ONE sync wait per Matmult: funnel every matmul
            # input through the vector engine so PE waits on a single DVE sem.
            nc.vector.tensor_copy(sb_d[:, :], sb_d16[:, :])
            nc.scalar.activation(w_act[:, :], sb_th16[:, :], mybir.ActivationFunctionType.Exp)
            nc.vector.tensor_copy(w_sb[:, :], w_act[:, :])
            nc.vector.memset(ones_row[:, :], 1.0)
            nc.vector.tensor_copy(row_di[:, :], row_di16[:, :])
            nc.vector.tensor_copy(row_thi[:, :], row_thi16[:, :])
            nc.vector.tensor_copy(row_ei[:, :], row_ei16[:, :])
            for t in range(NF):
                bc_ps = bcpool.tile([P, FT], F32, tag="bc")
                nc.tensor.matmul(
                    bc_ps[:, :], lhsT=ones_row[:, :],
                    rhs=row_di[:, t * FT:(t + 1) * FT], start=True, stop=True,
                )
                nc.vector.tensor_copy(bc_di[:, t * FT:(t + 1) * FT], bc_ps[:, :])

            # ---- main loop: mask gen + masked reduce ----
            risk_ps = [ppool.tile([1, FT], F32, name=f"risk{t}") for t in range(NF)]
            for c in range(NCH):
                mask_d = mpool.tile([P, NI], F32, tag="mask_d", name=f"mask_d{c}")
                nc.vector.tensor_scalar(
                    mask_d[:, :], bc_di[:, :],
                    sb_d[:, c:c + 1], None, mybir.AluOpType.is_le,
                )
                for t in range(NF):
                    nc.tensor.matmul(
                        risk_ps[t][:, :], lhsT=w_sb[:, c:c + 1],
                        rhs=mask_d[:, t * FT:(t + 1) * FT],
                        start=(c == 0), stop=(c == NCH - 1),
                    )

            # ---- epilogue ----
            for t in range(NF):
                nc.vector.tensor_copy(risk_row[:, t * FT:(t + 1) * FT],
                                      risk_ps[t][:, :])

            # (tensor_tensor_reduce crashes at runtime on this stack — use
            # separate mul + reduce_sum instead)
            nc.scalar.activation(ln_row[:, :], risk_row[:, :],
                                 mybir.ActivationFunctionType.Ln)
            nc.vector.tensor_sub(diff_row[:, :], row_thi[:, :], ln_row[:, :])
            nc.vector.tensor_mul(prod_row[:, :], diff_row[:, :], row_ei[:, :])
            nc.vector.reduce_sum(part_sb[:, :], prod_row[:, :],
                                 axis=mybir.AxisListType.X)
            nc.sync.dma_start(out=out[:, :], in_=part_sb[:, :])

    nc.finalize()
    return nc


# ---------------------------------------------------------------------------
# Cached PJRT dispatch.
#
# run_bass_kernel_spmd builds a FRESH jax.jit(shard_map(...)) on every call,
# which re-traces and re-lowers each time (~150 ms of host work per launch).
# We replicate its axon path here but hoist the jit construction into a
# once-per-process cache, so a steady-state call is: queue one host->device
# blob transfer + one launch, then block on the 32-byte partials fetch.
# Outputs are NOT passed as donated zero buffers (the run_bass_kernel_spmd
# scheme for kernels that underwrite their outputs): this kernel writes its
# whole [1,1] output, so the uninit PJRT result buffer is fine and we skip
# one host->device transfer per call.
# ---------------------------------------------------------------------------

_STATE = None


def _get_state():
    global _STATE
    if _STATE is not None:
        return _STATE

    import jax
    from concourse import bass2jax
    from jax.experimental.shard_map import shard_map
    from jax.sharding import Mesh, PartitionSpec

    nc = _build()
    bass2jax.install_neuronx_cc_hook()
    assert nc.dbg_addr is None

    partition_name = nc.partition_id_tensor.name if nc.partition_id_tensor else None

    in_names, out_names, out_avals = [], [], []
    for alloc in nc.m.functions[0].allocations:
        if not isinstance(alloc, mybir.MemoryLocationSet):
            continue
        name = alloc.memorylocations[0].name
        if alloc.kind == "ExternalInput":
            if name != partition_name:
                in_names.append(name)
        elif alloc.kind == "ExternalOutput":
            out_names.append(name)
            shape = tuple(alloc.tensor_shape)
            dtype = mybir.dt.np(alloc.dtype)
            out_avals.append(jax.core.ShapedArray(shape, dtype))
    all_names = list(in_names)
    if partition_name is not None:
        all_names.append(partition_name)

    def _body(*args):
        operands = list(args)
        if partition_name is not None:
            operands.append(bass2jax.partition_id_tensor())
        outs = bass2jax._bass_exec_p.bind(
            *operands,
            out_avals=tuple(out_avals),
            in_names=tuple(all_names),
            out_names=tuple(out_names),
            lowering_input_output_aliases=(),
            sim_require_finite=True,
            sim_require_nnan=True,
            nc=nc,
        )
        return tuple(outs)

    devices = jax.devices()[:NCORES]
    assert len(devices) == NCORES, f"need {NCORES} devices, have {len(jax.devices())}"
    mesh = Mesh(np.asarray(devices), ("core",))
    in_specs = (PartitionSpec("core"),) * len(in_names)
    out_specs = (PartitionSpec("core"),) * len(out_names)

    def _fresh_jit():
        return jax.jit(
            shard_map(_body, mesh=mesh, in_specs=in_specs, out_specs=out_specs,
                      check_rep=False),
            keep_unused=True,
        )

    # AOT-compile with the bass effect suppressed so calls take JAX's C++
    # fast-dispatch path (~1.5 ms less host work per call than the effectful
    # python dispatch). Falls back to the plain jit if the fast path is
    # unavailable on this jax version.
    blob_aval = jax.core.ShapedArray((NCORES * BLOB,), np.float16)
    try:
        sharded = bass2jax.fast_dispatch_compile(
            lambda: _fresh_jit().lower(blob_aval).compile())
    except Exception:
        sharded = _fresh_jit()

    _STATE = (sharded, in_names, out_names, out_avals)
    return _STATE


def _make_blob(hazard_pred, durations, events):
    theta = np.asarray(hazard_pred, dtype=np.float32).reshape(-1).astype(np.float16)
    d = np.asarray(durations, dtype=np.float32).astype(np.float16)
    e = np.asarray(events, dtype=np.float32).astype(np.float16)
    blob = np.empty((NCORES, BLOB), np.float16)
    if GATHER:
        blob[:, 0:NI] = d.reshape(NCORES, NI)
        blob[:, NI:2 * NI] = theta.reshape(NCORES, NI)
        blob[:, 2 * NI:] = e.reshape(NCORES, NI)
    else:
        blob[:, 0:N] = d
        blob[:, N:2 * N] = theta
        blob[:, 2 * N:2 * N + NI] = d.reshape(NCORES, NI)
        blob[:, 2 * N + NI:2 * N + 2 * NI] = theta.reshape(NCORES, NI)
        blob[:, 2 * N + 2 * NI:] = e.reshape(NCORES, NI)
    return blob.reshape(NCORES * BLOB)


def kernel(hazard_pred, durations, events):
    sharded, in_names, out_names, out_avals = _get_state()
    blob = _make_blob(hazard_pred, durations, events)
    out_arrs = sharded(blob)
    parts = np.asarray(out_arrs[0]).reshape(-1)
    loss = -(np.sum(parts.astype(np.float64)) / N)
    return np.asarray(loss, dtype=np.float32)


def run(hazard_pred, durations, events, trace=False, dve_cols=None, col_tile=None):
    # kept for test.py compatibility; trace/dve_cols/col_tile are ignored
    return kernel(hazard_pred, durations, events), None
